# revision 16
# baseline (speedup 1.0000x reference)
"""Trainium2 Bass kernel for the AdapterModel problem (v2).

Data-parallel over batch: core b computes pred[b] = f(seq_embed[b], aa_embed[b], ...).
No collectives (B == n_cores == 8); host gathers per-core outputs.

v2 design (vs v1):
  - Host pre-transposes the embeddings (seq^T [3072,896], aa^T [1280,512]) and
    pre-permutes the weights into eb-contiguous [128, (eb kb e')] layout, all in
    bf16.  This removes the entire on-device transpose phase (208 PE transposes
    + 208 DVE copies) and halves DMA bytes.
  - All hot matmuls run in bf16: fixes the f32r K<128 half-rate penalty that
    made the K=64 interaction matmuls run at 427ns instead of ~216ns.
  - Norms: nsq computed directly in [i, 2] / [j, 2] layout by using the squared
    latents as the *stationary* operand against a tiny pairmask moving operand,
    eliminating the [2, N] PSUM->SBUF copies and the ibat transposes.
  - Software pipeline: interaction MMs + EXPs of e-block eb-1 are interleaved
    between the projection MMs of eb so the in-order PE queue never stalls on
    the (slower) ACT exp+accum consumer; seq-nsq MMs of eb-1 are likewise
    emitted at the start of iteration eb.
  - All weights resident in SBUF (~116KB/partition), DMA'd once up front.

Math per core (N=896, J=512, H=32, D=64), same as v1:
  s[i,h] = sum_j exp(<seq_h[:,i], aa_nrm_h[:,j]> * (100*rsqrt(nsq_i)) - 40)
  r[i,h] = 0.01*ln(s) + 0.01*(40 - 2*ln n)
  out[i] = softplus(r[i,:] @ wv + pred_b),  wv = (to_logits_w .* sigmoid(ctx@ctx_w)) @ pred_w
"""
import sys

if "/opt/trn_rl_repo" not in sys.path:
    sys.path.insert(0, "/opt/trn_rl_repo")

import math
import numpy as np

H = 32
D = 64
E = H * D            # 2048
SEQ_D = 3072
AA_D = 1280
CTX_D = 768
B, N, J = 8, 896, 512
KS = SEQ_D // 128    # 24
KA = AA_D // 128     # 10
EB = E // 128        # 16
IB = N // 128        # 7
JB = J // 128        # 4
NH = N // 2          # 448
KCTX = CTX_D // 128  # 6
MAGIC = 0x5F3759DF

# how many seq-proj MMs between interleaved interaction jobs
INT_STRIDE = 3

_GRAPH_CACHE = {}


def _build(pred_b_val: float):
    key = (float(pred_b_val),)
    if key in _GRAPH_CACHE:
        return _GRAPH_CACHE[key]

    import concourse.bacc as bacc
    import concourse.mybir as mybir
    import concourse.tile as tile

    F32 = mybir.dt.float32
    F32R = mybir.dt.float32r
    BF16 = mybir.dt.bfloat16
    F8 = mybir.dt.float8e4
    U32 = mybir.dt.uint32
    AF = mybir.ActivationFunctionType
    AL = mybir.AluOpType

    nc = bacc.Bacc("TRN2", target_bir_lowering=False, debug=False, num_devices=8)

    FP8 = None  # set below
    seqT_ext = nc.dram_tensor("seqT8", [SEQ_D // 256 * 128, 2 * N], mybir.dt.float8e4, kind="ExternalInput")
    aaT_ext = nc.dram_tensor("aaT8", [AA_D // 256 * 128, 2 * J], mybir.dt.float8e4, kind="ExternalInput")
    seqw_ext = nc.dram_tensor("seq_wp8", [128, EB * KS * 128], mybir.dt.float8e4, kind="ExternalInput")
    aaw_ext = nc.dram_tensor("aa_wp8", [128, EB * KA * 128], mybir.dt.float8e4, kind="ExternalInput")
    seqb_ext = nc.dram_tensor("seq_b2", [128, EB], F32, kind="ExternalInput")
    aab_ext = nc.dram_tensor("aa_b2", [128, EB], F32, kind="ExternalInput")
    ctxT_ext = nc.dram_tensor("ctxT", [128, KCTX], BF16, kind="ExternalInput")
    ctxw_ext = nc.dram_tensor("ctx_wp", [CTX_D, H * H], BF16, kind="ExternalInput")
    ctxb_ext = nc.dram_tensor("ctx_bp", [128, 8], F32, kind="ExternalInput")
    tlw_ext = nc.dram_tensor("tlwT", [128, 8], F32, kind="ExternalInput")
    predw_ext = nc.dram_tensor("pred_w", [H, 1], F32, kind="ExternalInput")
    maskT_ext = nc.dram_tensor("maskT", [128, 2 * JB], F32, kind="ExternalInput")
    cvec_ext = nc.dram_tensor("cvec", [128, 1], F32, kind="ExternalInput")
    eye_ext = nc.dram_tensor("eye128", [128, 128], F32, kind="ExternalInput")
    pm_ext = nc.dram_tensor("pairmask", [128, 2], BF16, kind="ExternalInput")
    sel2_ext = nc.dram_tensor("sel2", [2, 128], BF16, kind="ExternalInput")
    ones1_ext = nc.dram_tensor("ones1", [1, 128], F32, kind="ExternalInput")
    out_ext = nc.dram_tensor("out", [N], F32, kind="ExternalOutput")

    with tile.TileContext(nc) as tc:
        with tc.tile_pool(name="persist", bufs=1) as pp, \
             tc.tile_pool(name="work", bufs=2) as wp, \
             tc.tile_pool(name="pa", bufs=1, space="PSUM") as pa, \
             tc.tile_pool(name="psq", bufs=1, space="PSUM") as psq, \
             tc.tile_pool(name="pips", bufs=4, space="PSUM") as pips, \
             tc.tile_pool(name="pn", bufs=1, space="PSUM") as pn:

            # ---- constants / tiny inputs ----
            eyer = pp.tile([128, 128], F32R, tag="eyer")
            pairmask = pp.tile([128, 2], BF16, tag="pairmask")
            nc.sync.dma_start(out=pairmask[:], in_=pm_ext[:])
            sel2 = pp.tile([2, 128], BF16, tag="sel2")
            nc.sync.dma_start(out=sel2[:], in_=sel2_ext[:])
            ones1 = pp.tile([1, 128], F32R, tag="ones1")
            maskT = pp.tile([128, 2 * JB], F32, tag="maskT")
            nc.sync.dma_start(out=maskT[:], in_=maskT_ext[:])
            cvec = pp.tile([128, 1], F32, tag="cvec")
            nc.sync.dma_start(out=cvec[:], in_=cvec_ext[:])
            seqb2 = pp.tile([128, EB], F32, tag="seqb2")
            nc.sync.dma_start(out=seqb2[:], in_=seqb_ext[:])
            aab2 = pp.tile([128, EB], F32, tag="aab2")
            nc.sync.dma_start(out=aab2[:], in_=aab_ext[:])
            # const bias columns: [0]=-40, [1]=1e-30
            cb = pp.tile([128, 2], F32, tag="cb")
            nc.gpsimd.memset(cb[:, 0:1], -40.0)
            nc.gpsimd.memset(cb[:, 1:2], 1e-30)
            magic = pp.tile([128, 16], U32, tag="magic")
            nc.gpsimd.memset(magic[:], MAGIC)

            def rsqrt_newton(name, xin_ap, w, scale_mul, out_ap):
                """out_ap (SBUF f32 [128, w]) <- rsqrt(xin * scale_mul)."""
                x = wp.tile([128, 16], F32, tag="nwx", name=f"nwx{name}")
                nc.vector.tensor_scalar(x[:, :w], xin_ap, scale_mul, 1e-35, op0=AL.mult, op1=AL.add)
                u = wp.tile([128, 16], U32, tag="nwu", name=f"nwu{name}")
                nc.vector.tensor_scalar(u[:, :w], x[:, :w].bitcast(U32), 1, None, op0=AL.logical_shift_right)
                y0u = wp.tile([128, 16], U32, tag="nwy0", name=f"nwy0{name}")
                nc.vector.tensor_tensor(out=y0u[:, :w], in0=magic[:, :w], in1=u[:, :w], op=AL.subtract)
                t = wp.tile([128, 16], F32, tag="nwt", name=f"nwt{name}")
                nc.vector.tensor_mul(t[:, :w], y0u[:, :w].bitcast(F32), y0u[:, :w].bitcast(F32))
                nc.vector.tensor_mul(t[:, :w], t[:, :w], x[:, :w])
                nc.vector.tensor_scalar(t[:, :w], t[:, :w], -0.5, 1.5, op0=AL.mult, op1=AL.add)
                y = wp.tile([128, 16], F32, tag="nwy", name=f"nwy{name}")
                nc.vector.tensor_mul(y[:, :w], y0u[:, :w].bitcast(F32), t[:, :w])
                nc.vector.tensor_mul(t[:, :w], y[:, :w], y[:, :w])
                nc.vector.tensor_mul(t[:, :w], t[:, :w], x[:, :w])
                nc.vector.tensor_scalar(t[:, :w], t[:, :w], -0.5, 1.5, op0=AL.mult, op1=AL.add)
                nc.vector.tensor_mul(out_ap, y[:, :w], t[:, :w])

            # ---- gating head chain (emitted later, mid-loop: its DMAs must
            # never block the hot-loop PE queue head) ----
            ctxT = pp.tile([128, KCTX], BF16, tag="ctxT")
            nc.sync.dma_start(out=ctxT[:], in_=ctxT_ext[:])
            ctxb2 = pp.tile([128, 8], F32, tag="ctxb2")
            nc.sync.dma_start(out=ctxb2[:], in_=ctxb_ext[:])
            tlw2 = pp.tile([128, 8], F32, tag="tlw2")
            nc.sync.dma_start(out=tlw2[:], in_=tlw_ext[:])
            wctxs = [pp.tile([128, H * H], BF16, tag=f"wctx{c}", name=f"wctx{c}") for c in range(KCTX)]
            g_dram = nc.dram_tensor("g_bounce", [H * H], F32)
            wb_dram = nc.dram_tensor("wb_bounce", [H, H], F32)
            wbT = pp.tile([H, H], F32R, tag="wbT")
            predw = pp.tile([H, 1], F32R, tag="predw")
            WV224 = pp.tile([128, IB * H], F32, tag="WV224")

            def gating_head():
                g_ps = [pips.tile([1, 512], F32, tag="ips", name=f"gps{i}") for i in range(2)]
                for c in range(KCTX):
                    for half in range(2):
                        nc.tensor.matmul(g_ps[half][:], ctxT[:, c:c + 1], wctxs[c][:, 512 * half:512 * (half + 1)],
                                         start=(c == 0), stop=(c == KCTX - 1))
                g_sb = pp.tile([1, H * H], F32, tag="g_sb")
                for half in range(2):
                    nc.vector.tensor_copy(g_sb[:, 512 * half:512 * (half + 1)], g_ps[half][:])
                nc.sync.dma_start(out=g_dram.ap()[None, :], in_=g_sb[:])
                g2 = pp.tile([128, 8], F32, tag="g2")
                nc.sync.dma_start(out=g2[:], in_=g_dram.ap().rearrange("(c p) -> p c", p=128))
                nc.vector.tensor_add(g2[:], g2[:], ctxb2[:])
                eg = pp.tile([128, 8], F32, tag="eg")
                nc.scalar.activation(eg[:], g2[:], AF.Exp, bias=0.0, scale=-1.0)
                nc.vector.tensor_scalar_add(eg[:], eg[:], 1.0)
                nc.vector.reciprocal(eg[:], eg[:])
                nc.vector.tensor_mul(eg[:], eg[:], tlw2[:])   # w_b, flat e-major = c*128+p
                nc.sync.dma_start(out=wb_dram.ap().rearrange("(a b) h -> (b h) a", a=8), in_=eg[:])
                nc.gpsimd.dma_start(out=wbT[:], in_=wb_dram[:])

            def gating_tail():
                # emitted after the hot loop: the DRAM bounce + sigmoid chain
                # must never stall hot-loop PE work
                wv_ps = pips.tile([1, H], F32, tag="ips")
                nc.tensor.matmul(wv_ps[:], predw[:], wbT[:], start=True, stop=True)
                wv_sb = pp.tile([1, H], F32R, tag="wv_sb")
                nc.vector.tensor_copy(wv_sb[:], wv_ps[:])
                WV_ps = pips.tile([128, H], F32, tag="ips")
                nc.tensor.matmul(WV_ps[:], ones1[:], wv_sb[:], start=True, stop=True)
                for ib in range(IB):
                    nc.vector.tensor_copy(WV224[:, H * ib:H * (ib + 1)], WV_ps[:])

            # ---- bulk DMA: everything resident ----
            KAP = KA // 2
            waas = [pp.tile([128, KA * 128], F8, tag=f"waa{eb}", name=f"waa{eb}") for eb in range(EB)]
            nc.gpsimd.dma_start(out=waas[0][:], in_=aaw_ext[:, 0:KA * 128])
            aaT = [pp.tile([128, 2, J], F8, tag=f"aat{k}", name=f"aat{k}") for k in range(KAP)]
            for k in range(KAP):
                nc.gpsimd.dma_start(out=aaT[k][:].rearrange("p a b -> p (a b)"),
                                    in_=aaT_ext[128 * k:128 * (k + 1), :])

            def load_ws(eb):
                # [128, (kp, ko, e')] fp8: per kb-pair kp a [128, 2, 128] DoubleRow weight block
                ws = wp.tile([128, KS * 128], F8, tag="ws", name=f"ws{eb}")
                nc.gpsimd.dma_start(out=ws[:], in_=seqw_ext[:, KS * 128 * eb:KS * 128 * (eb + 1)])
                return ws

            wss = {0: load_ws(0)}
            # late-needed consts, after the eb0-critical loads
            nc.gpsimd.dma_start(out=eyer[:], in_=eye_ext[:])
            nc.gpsimd.dma_start(out=ones1[:], in_=ones1_ext[:])
            nc.gpsimd.dma_start(out=predw[:], in_=predw_ext[:])
            # seqT + gating weights issue from the (idle) scalar engine so their
            # ~630ns/DMA descriptor-gen overlaps the gpsimd issues
            KP = KS // 2
            seqT = [pp.tile([128, 2, N], F8, tag=f"ast{kp}", name=f"ast{kp}") for kp in range(KP)]
            for kp in range(KP):
                nc.scalar.dma_start(out=seqT[kp][:].rearrange("p a b -> p (a b)"),
                                    in_=seqT_ext[128 * kp:128 * (kp + 1), :])
            for c in range(KCTX):
                nc.scalar.dma_start(out=wctxs[c][:], in_=ctxw_ext[128 * c:128 * (c + 1), :])
            for eb in range(1, EB):
                nc.gpsimd.dma_start(out=waas[eb][:], in_=aaw_ext[:, KA * 128 * eb:KA * 128 * (eb + 1)])

            # ---- hot loop ----
            s_t = pp.tile([128, IB * H], F32, tag="s_t")

            # per-iteration state carried across the software pipeline
            prev = {}   # eb-1 state: seq_sb, aa_nrm, invs, seq_sq

            def emit_seq_nsq(st):
                # nsq[i, hh] for eb-1 from its seq_sq (stationary) x pairmask
                nsq_ps = pn.tile([128, 16], F32, tag="nsq", name=f"nsqs{st['eb']}")
                for ib in range(IB):
                    nc.tensor.matmul(nsq_ps[:, 2 * ib:2 * ib + 2],
                                     st["seq_sq"][:, 128 * ib:128 * (ib + 1)],
                                     pairmask[:], start=True, stop=True)
                invs = wp.tile([128, 16], F32, tag="invs", name=f"invs{st['eb']}")
                rsqrt_newton(f"s{st['eb']}", nsq_ps[:, :2 * IB], 2 * IB, 1e-4, invs[:, :2 * IB])
                st["invs"] = invs

            def int_jobs_for(st):
                jobs = []
                eb_prev = st["eb"]
                for ib in range(IB):
                    for hh in range(2):
                        def job(ib=ib, hh=hh, st=st):
                            h = 2 * st["eb"] + hh
                            col = s_t[:, H * ib + h:H * ib + h + 1]
                            int_ps = pips.tile([128, J], F32, tag="ips", name=f"int{h}_{ib}")
                            nc.tensor.matmul(int_ps[:],
                                             st["seq_sb"][64 * hh:64 * (hh + 1), 128 * ib:128 * (ib + 1)],
                                             st["aa_nrm"][64 * hh:64 * (hh + 1), :],
                                             start=True, stop=True)
                            if hh == 0:
                                nc.scalar.activation(int_ps[:], int_ps[:], AF.Exp,
                                                     bias=cb[:, 0:1],
                                                     scale=st["invs"][:, 2 * ib + hh:2 * ib + hh + 1],
                                                     accum_out=col)
                            else:
                                # exp -> SBUF bf16, j-sum on DVE/GpSimd: keeps the
                                # ACT pipe free of READ_ACCUMULATOR overhead
                                exp_sb = wp.tile([128, J], BF16, tag="expsb", bufs=4, name=f"expsb{h}_{ib}")
                                nc.scalar.activation(exp_sb[:], int_ps[:], AF.Exp,
                                                     bias=cb[:, 0:1],
                                                     scale=st["invs"][:, 2 * ib + hh:2 * ib + hh + 1])
                                nc.vector.reduce_sum(col, exp_sb[:], axis=mybir.AxisListType.X)
                        jobs.append(job)
                return jobs

            for eb in range(EB):
                cur = {"eb": eb}
                if eb == 3:
                    gating_head()
                if eb + 1 < EB:
                    wss[eb + 1] = load_ws(eb + 1)   # prefetch next e-block's weights
                jobs = []
                if prev:
                    emit_seq_nsq(prev)          # seq-nsq MMs of eb-1 (needs seq_sq(eb-1))
                    jobs = int_jobs_for(prev)   # interaction MM+EXP closures of eb-1

                # aa projection for this e-block
                aa_ps = pa.tile([128, J], F32, tag="aa_ps", name=f"aaps{eb}")
                for k in range(KAP):
                    nc.tensor.matmul(aa_ps[:], waas[eb][:, 256 * k:256 * (k + 1)].rearrange("p (a b) -> p a b", a=2),
                                     aaT[k][:],
                                     start=(k == 0), stop=(k == KAP - 1),
                                     perf_mode=mybir.MatmulPerfMode.DoubleRow)
                aa_raw = wp.tile([128, J], BF16, tag="araw", name=f"araw{eb}")
                nc.vector.tensor_scalar_add(aa_raw[:], aa_ps[:], aab2[:, eb:eb + 1])
                aa_sq = wp.tile([128, J], BF16, tag="aasq", name=f"aasq{eb}")
                nc.vector.tensor_mul(aa_sq[:], aa_raw[:], aa_raw[:])

                # seq projection, with eb-1 interaction jobs + aa-norm machinery
                # interleaved into the PE stream
                sq_ps = [psq.tile([128, NH], F32, tag=f"sq{c}", name=f"sqps{eb}_{c}") for c in range(2)]
                nmm = 0
                ji = 0

                def pump():
                    nonlocal ji
                    if ji < len(jobs) and nmm >= INT_STRIDE * (ji // 2 + 1):
                        jobs[ji]()
                        jobs[ji + 1]()
                        ji += 2

                for kp in range(KP):
                    wsl = wss[eb][:, 256 * kp:256 * (kp + 1)].rearrange("p (a b) -> p a b", a=2)
                    for c in range(2):
                        nc.tensor.matmul(sq_ps[c][:], wsl,
                                         seqT[kp][:, :, NH * c:NH * (c + 1)],
                                         start=(kp == 0), stop=(kp == KP - 1),
                                         perf_mode=mybir.MatmulPerfMode.DoubleRow)
                        nmm += 1
                        pump()
                    if kp == 4:
                        # aa nsq: [j, hh] via aa_sq stationary x pairmask
                        nsqa_ps = pn.tile([128, 16], F32, tag="nsq", name=f"nsqa{eb}")
                        for jb in range(JB):
                            nc.tensor.matmul(nsqa_ps[:, 2 * jb:2 * jb + 2],
                                             aa_sq[:, 128 * jb:128 * (jb + 1)],
                                             pairmask[:], start=True, stop=True)
                        ya = wp.tile([128, 16], F32R, tag="ya", name=f"ya{eb}")
                        rsqrt_newton(f"a{eb}", nsqa_ps[:, :2 * JB], 2 * JB, 1.0, ya[:, :2 * JB])
                        nc.vector.tensor_mul(ya[:, :2 * JB], ya[:, :2 * JB], maskT[:])
                    if kp == 6:
                        # transpose ya -> [2, J] rows, bounce via ACT to bf16
                        rowa_ps = pips.tile([2, J], F32R, tag="ips", name=f"rowa{eb}")
                        for jb in range(JB):
                            nc.tensor.transpose(rowa_ps[:, 128 * jb:128 * (jb + 1)], ya[:, 2 * jb:2 * jb + 2], eyer[:])
                        inva = wp.tile([2, J], BF16, tag="inva", name=f"inva{eb}")
                        nc.vector.tensor_copy(inva[:], rowa_ps[:].bitcast(F32))
                    if kp == 8:
                        # broadcast inv-norms to 128 partitions, normalize aa
                        bc_ps = pa.tile([128, J], F32, tag="aa_ps", name=f"bc{eb}")
                        nc.tensor.matmul(bc_ps[:], sel2[:], inva[:], start=True, stop=True)
                        aa_nrm = wp.tile([128, J], BF16, tag="aanrm", name=f"aanrm{eb}")
                        nc.vector.tensor_mul(aa_nrm[:], aa_raw[:], bc_ps[:])
                        cur["aa_nrm"] = aa_nrm

                while ji < len(jobs):
                    jobs[ji]()
                    ji += 1

                seq_sb = wp.tile([128, N], BF16, tag="seq_sb", name=f"seqsb{eb}")
                for c in range(2):
                    nc.vector.tensor_scalar_add(seq_sb[:, NH * c:NH * (c + 1)], sq_ps[c][:], seqb2[:, eb:eb + 1])
                seq_sq = wp.tile([128, N], BF16, tag="seqsq", name=f"seqsq{eb}")
                nc.vector.tensor_mul(seq_sq[:], seq_sb[:], seq_sb[:])
                cur["seq_sb"] = seq_sb
                cur["seq_sq"] = seq_sq
                prev = cur

            # drain the pipeline: eb=15's nsq + interactions
            emit_seq_nsq(prev)
            gating_tail()
            for job in int_jobs_for(prev):
                job()

            # ---- phase 2: r = 0.01*ln(s)+cval; out = softplus(r @ wv + pred_b) ----
            r1 = wp.tile([128, IB * H], F32, tag="r1", bufs=1)
            nc.scalar.activation(r1[:], s_t[:], AF.Ln, bias=cb[:, 1:2], scale=1.0)
            nc.vector.tensor_scalar(r1[:], r1[:], 0.01, cvec[:, 0:1], op0=AL.mult, op1=AL.add)
            nc.vector.tensor_mul(r1[:], r1[:], WV224[:])
            pps = wp.tile([128, IB], F32, tag="pp_t", bufs=1)
            for ib in range(IB):
                nc.vector.reduce_sum(pps[:, ib:ib + 1], r1[:, H * ib:H * (ib + 1)], axis=mybir.AxisListType.X)
            nc.vector.tensor_scalar(pps[:], pps[:], float(pred_b_val), 80.0, op0=AL.add, op1=AL.min)
            # softplus(z) = max(z,0) + ln(1 + 2^(-|z|*log2e)); 2^f via DVE
            # bit-trick so ACT only needs the already-loaded natural-log set
            I32 = mybir.dt.int32
            rmax = wp.tile([128, IB], F32, tag="rmax", bufs=1)
            nc.vector.tensor_scalar_max(rmax[:], pps[:], 0.0)
            a = wp.tile([128, IB], F32, tag="spa", bufs=1)
            nc.vector.tensor_scalar_mul(a[:], pps[:], -1.0)
            nc.vector.tensor_tensor(out=a[:], in0=a[:], in1=pps[:], op=AL.max)
            t = wp.tile([128, IB], F32, tag="spt", bufs=1)
            nc.vector.tensor_scalar_mul(t[:], a[:], -1.4426950408889634)
            ki = wp.tile([128, IB], I32, tag="spk", bufs=1)
            nc.vector.tensor_copy(ki[:], t[:])
            kf = wp.tile([128, IB], F32, tag="spkf", bufs=1)
            nc.vector.tensor_copy(kf[:], ki[:])
            frac = wp.tile([128, IB], F32, tag="spf", bufs=1)
            nc.vector.tensor_tensor(out=frac[:], in0=t[:], in1=kf[:], op=AL.subtract)
            EXP2C = [0.9999999892448939, 0.6931471766184188, 0.2402268578559816,
                     0.0555041610602884, 0.009616380954260166, 0.0013331706940601918,
                     0.00015677647395067208, 1.54920373946812e-05]
            acc = wp.tile([128, IB], F32, tag="spacc", bufs=1)
            nc.vector.tensor_scalar(acc[:], frac[:], EXP2C[7], EXP2C[6], op0=AL.mult, op1=AL.add)
            for c in range(5, -1, -1):
                nc.vector.tensor_mul(acc[:], acc[:], frac[:])
                nc.vector.tensor_scalar_add(acc[:], acc[:], EXP2C[c])
            nc.vector.tensor_scalar_add(ki[:], ki[:], 127)
            nc.vector.tensor_scalar(ki[:], ki[:], 23, None, op0=AL.logical_shift_left)
            nc.vector.tensor_mul(acc[:], acc[:], ki[:].bitcast(F32))
            lnw = wp.tile([128, IB], F32, tag="splnw", bufs=1)
            nc.scalar.activation(lnw[:], acc[:], AF.Ln, bias=1.0, scale=1.0)
            nc.vector.tensor_add(pps[:], rmax[:], lnw[:])
            for ib in range(IB):
                nc.sync.dma_start(out=out_ext[128 * ib:128 * (ib + 1)], in_=pps[:, ib:ib + 1])

    nc.compile()
    _GRAPH_CACHE[key] = nc
    return nc


def _prep_in_maps(inputs):
    import ml_dtypes
    BF = ml_dtypes.bfloat16
    F8 = ml_dtypes.float8_e4m3

    seq_embed = np.asarray(inputs["seq_embed"], dtype=np.float32)
    aa_embed = np.asarray(inputs["aa_embed"], dtype=np.float32)
    ctx = np.asarray(inputs["contextual_embed"], dtype=np.float32)
    aa_mask = np.asarray(inputs["aa_mask"])
    seq_w = np.asarray(inputs["seq_w"], dtype=np.float32)
    seq_b = np.asarray(inputs["seq_b"], dtype=np.float32)
    aa_w = np.asarray(inputs["aa_w"], dtype=np.float32)
    aa_b = np.asarray(inputs["aa_b"], dtype=np.float32)
    tlw = np.asarray(inputs["to_logits_w"], dtype=np.float32)
    ctx_w = np.asarray(inputs["ctx_w"], dtype=np.float32)
    ctx_b = np.asarray(inputs["ctx_b"], dtype=np.float32)
    pred_w = np.ascontiguousarray(inputs["pred_w"], dtype=np.float32)

    # seq weights: [128ki, (eb kp ko e')] fp8 for DoubleRow kb-pair contraction
    seq_wp = np.ascontiguousarray(
        seq_w.reshape(KS // 2, 2, 128, EB, 128).transpose(2, 3, 0, 1, 4).reshape(128, EB * KS * 128)).astype(F8)
    aa_wp_ = np.ascontiguousarray(
        aa_w.reshape(KA // 2, 2, 128, EB, 128).transpose(2, 3, 0, 1, 4).reshape(128, EB * KA * 128)).astype(F8)

    # gating space permuted h-major -> e-major
    perm = (np.arange(H * H).reshape(H, H).T).reshape(-1)
    ctx_wp = np.ascontiguousarray(ctx_w[:, perm]).astype(BF)
    # gating-space vectors in [128, 8] layout: flat e-major index j = c*128 + p
    ctx_bp = np.ascontiguousarray(ctx_b[perm].reshape(8, 128).T)
    tlwT = np.ascontiguousarray(tlw.T.reshape(8, 128).T)

    seq_b2 = np.ascontiguousarray(seq_b.reshape(EB, 128).T)
    aa_b2 = np.ascontiguousarray(aa_b.reshape(EB, 128).T)
    eye128 = np.eye(128, dtype=np.float32)
    pairmask = np.zeros((128, 2), dtype=np.float32)
    pairmask[:64, 0] = 1.0
    pairmask[64:, 1] = 1.0
    pairmask = pairmask.astype(BF)
    sel2 = np.zeros((2, 128), dtype=np.float32)
    sel2[0, :64] = 1.0
    sel2[1, 64:] = 1.0
    sel2 = sel2.astype(BF)
    ones1 = np.ones((1, 128), dtype=np.float32)

    in_maps = []
    for b in range(B):
        m = aa_mask[b].astype(np.float32)
        n_b = max(float(m.sum()), 1.0)
        cval = 0.01 * (40.0 - 2.0 * math.log(n_b))  # reference's logavgexp subtracts ln n twice
        mT = np.zeros((128, 2 * JB), dtype=np.float32)
        for c in range(JB):
            mT[:, 2 * c] = m[128 * c:128 * (c + 1)]
            mT[:, 2 * c + 1] = m[128 * c:128 * (c + 1)]
        in_maps.append({
            "seqT8": np.ascontiguousarray(
                seq_embed[b].T.reshape(KS // 2, 2, 128, N).transpose(0, 2, 1, 3).reshape(KS // 2 * 128, 2 * N)).astype(F8),
            "aaT8": np.ascontiguousarray(
                aa_embed[b].T.reshape(KA // 2, 2, 128, J).transpose(0, 2, 1, 3).reshape(KA // 2 * 128, 2 * J)).astype(F8),
            "seq_wp8": seq_wp,
            "aa_wp8": aa_wp_,
            "seq_b2": seq_b2,
            "aa_b2": aa_b2,
            "ctxT": np.ascontiguousarray(ctx[b].reshape(KCTX, 128).T).astype(BF),
            "ctx_wp": ctx_wp,
            "ctx_bp": ctx_bp,
            "tlwT": tlwT,
            "pred_w": pred_w,
            "maskT": mT,
            "cvec": np.full((128, 1), cval, dtype=np.float32),
            "eye128": eye128,
            "pairmask": pairmask,
            "sel2": sel2,
            "ones1": ones1,
        })
    return in_maps


def _run(inputs, trace=False, n_cores=B):
    from concourse.bass_utils import run_bass_kernel_spmd
    pred_b_val = float(np.asarray(inputs["pred_b"]).reshape(-1)[0])
    nc = _build(pred_b_val)
    in_maps = _prep_in_maps(inputs)
    res = run_bass_kernel_spmd(nc, in_maps[:n_cores], core_ids=list(range(n_cores)), trace=trace)
    out = np.stack([res.results[c]["out"] for c in range(n_cores)], axis=0)
    return out, res


def kernel(**inputs) -> np.ndarray:
    out, _ = _run(inputs, trace=False)
    return out
